# revision 28
# baseline (speedup 1.0000x reference)
# GGNN encoder kernel for Trainium2 (Bass/Tile), data-parallel over the
# batch dimension: 8 graphs -> 8 NeuronCores, one graph per core.
#
# Per-core computation (one graph):
#   type_e  = type_table[node_types]                       # [N, TD]
#   tok_e   = word_emb[node_token_ids]                     # [T, D]   (SWDGE dma_gather)
#   text_e  = segment_mean(tok_e, token_seg_ids)           # [N, D]   (PE matmul w/ pooling matrix)
#   h       = concat(type_e, text_e) @ fusion_w + b        # [N, D]
#   4 x GGNN layer:
#     m    = h @ Wl                                        # [N, D]
#     agg  = A @ m          (A dense adjacency, built host-side from edge list)
#     GRU(h, agg)
#   out     = mask * h
#
# Layout strategy: h, agg, gates are kept feature-major ("T" layout,
# [feat partitions, node free-dim]) so that the feature-contracting GRU
# matmuls can run directly; m is node-major for the node-contracting
# scatter matmul. Matmuls run as float32r (full fp32 storage, single-pass
# PE mode) for 4x throughput over plain fp32.

import functools

import numpy as np

import concourse.bass as bass
import concourse.mybir as mybir
import concourse.tile as tile
from concourse import bacc, bass_utils
from concourse.masks import make_identity

# Problem shapes (hardcoded: kernel must be self-contained).
B, N, T, D, TD, L = 8, 512, 2048, 768, 128, 4
V, TYPES = 30522, 64
MAX_NODE_LEN = 512
K3 = 3 * D            # 2304 stacked GRU gate rows
F = TD + D            # 896 fused embedding dim
P = 128               # partitions
NCH = N // P          # 4 node chunks
TCH = T // P          # 16 token chunks
DCH = D // P          # 6 feature chunks
FCH = F // P          # 7 fused-dim chunks
GCH = 3 * DCH         # 18 gate row chunks
BLK = N // TCH        # 32 nodes per token chunk (block-pooling case)
NF = 512              # free-dim tile (nodes)
GS = 4                # token gather splits
GT = T // GS          # tokens per gather split (512)
GC = GT // P          # 128-chunks per gather split (4)

f32 = mybir.dt.float32
f32r = mybir.dt.float32r
i32 = mybir.dt.int32
i16 = mybir.dt.int16

Sigmoid = mybir.ActivationFunctionType.Sigmoid
Tanh = mybir.ActivationFunctionType.Tanh
Ident = mybir.ActivationFunctionType.Identity


def build_nc(pool_wide: bool) -> bass.Bass:
    nc = bacc.Bacc(num_swdge_queues=2, dynamic_dma_scratch_size=32768)

    # All host-side tensors are pre-laid-out partition-major so every DMA is
    # contiguous per partition.
    tok_idx = nc.dram_tensor("tok_idx", [P, GS * (GT // 16)], i16,
                             kind="ExternalInput")  # [128, 4*32] wrapped idxs
    typ_idx = nc.dram_tensor("typ_idx", [P, N // 16], i16, kind="ExternalInput")
    word_emb = nc.dram_tensor("word_emb", [V, D], f32r, kind="ExternalInput")
    type_table = nc.dram_tensor("type_table", [TYPES, TD], f32, kind="ExternalInput")
    pool_w = N if pool_wide else BLK
    poolm = nc.dram_tensor("poolm", [P, TCH, pool_w], f32r, kind="ExternalInput")
    at_w = nc.dram_tensor("at_w", [P, NCH, N], f32r, kind="ExternalInput")
    fusion_w = nc.dram_tensor("fusion_w", [F, D], f32r, kind="ExternalInput")
    fusion_b = nc.dram_tensor("fusion_b", [P, DCH], f32, kind="ExternalInput")
    wl = nc.dram_tensor("wl", [L, DCH, P, D], f32r, kind="ExternalInput")
    wih = nc.dram_tensor("wih", [P, DCH, K3], f32r, kind="ExternalInput")
    whh_st = nc.dram_tensor("whh_st", [GCH, P, DCH, P], f32r, kind="ExternalInput")
    bsum = nc.dram_tensor("bsum", [P, GCH], f32, kind="ExternalInput")
    bihn = nc.dram_tensor("bihn", [P, DCH], f32, kind="ExternalInput")
    bhhn = nc.dram_tensor("bhhn", [P, DCH], f32, kind="ExternalInput")
    maskc = nc.dram_tensor("maskc", [P, NCH], f32, kind="ExternalInput")
    out = nc.dram_tensor("out", [N, D], f32, kind="ExternalOutput")

    with tile.TileContext(nc) as tc:
        with (
            tc.tile_pool(name="consts", bufs=1) as consts,
            tc.tile_pool(name="wbig", bufs=1) as wbig,
            tc.tile_pool(name="t768", bufs=7) as t768,
            tc.tile_pool(name="c512", bufs=7) as c512,
            tc.tile_pool(name="hpool", bufs=12) as hpool,
            tc.tile_pool(name="gpool", bufs=5) as gpool,
            tc.tile_pool(name="wst", bufs=3) as wst,
            tc.tile_pool(name="wlc", bufs=7) as wlc,
            tc.tile_pool(name="tokg", bufs=2) as tokg,
            tc.tile_pool(name="psA", bufs=7, space="PSUM") as psA,
        ):
            # ---- token gather first: it gates the whole front of the kernel
            tok_idx_sb = consts.tile([P, T // 16], i16)
            nc.sync.dma_start(out=tok_idx_sb[:], in_=tok_idx[:])
            pool_sb = consts.tile([P, TCH, pool_w], f32r)
            nc.sync.dma_start(out=pool_sb[:], in_=poolm[:])

            # type gather first: tiny transfer, unblocks the PE-queue head
            typ_idx_sb = consts.tile([P, N // 16], i16)
            nc.sync.dma_start(out=typ_idx_sb[:], in_=typ_idx[:])
            typg = consts.tile([P, NCH, TD], f32, tag="typg")
            nc.gpsimd.dma_gather(
                typg[:], type_table[:], typ_idx_sb[:], N, N, TD, queue_num=1
            )

            gath = []
            gath_insts = []
            for s in range(GS):
                tg = tokg.tile([P, GC, D], f32r, tag="tokg", name=f"tokg{s}")
                gi_ = nc.gpsimd.dma_gather(
                    tg[:],
                    word_emb[:],
                    tok_idx_sb[:, s * (GT // 16) : (s + 1) * (GT // 16)],
                    GT,
                    GT,
                    D,
                    queue_num=s % 2,
                )
                gath.append(tg)
                gath_insts.append(gi_)

            def after_gathers(dma_inst):
                return dma_inst

            # ---- remaining constants / small inputs ----
            identity = consts.tile([P, P], f32)
            make_identity(nc, identity[:])
            bsum_sb = consts.tile([P, GCH], f32)
            nc.sync.dma_start(out=bsum_sb[:], in_=bsum[:])
            bihn_sb = consts.tile([P, DCH], f32)
            nc.sync.dma_start(out=bihn_sb[:], in_=bihn[:])
            bhhn_sb = consts.tile([P, DCH], f32)
            nc.sync.dma_start(out=bhhn_sb[:], in_=bhhn[:])
            fb_sb = consts.tile([P, DCH], f32)
            nc.sync.dma_start(out=fb_sb[:], in_=fusion_b[:])
            mask_sb = consts.tile([P, NCH], f32)
            nc.sync.dma_start(out=mask_sb[:], in_=maskc[:])

            # ---- fused embedding (feature-major [f, n]) ----
            fusedT = [
                c512.tile([P, NF], f32r, tag="c512", name=f"fusedT{k}")
                for k in range(FCH)
            ]

            # weight loads, emitted in the order the compute will need them
            # (the DMA engines drain roughly in emission order)
            fw = []
            for k in range(FCH):
                fwk = t768.tile([P, D], f32r, tag="t768", name=f"fw{k}")
                after_gathers(nc.scalar.dma_start(
                    out=fwk[:], in_=fusion_w[k * P : (k + 1) * P, :]
                ))
                fw.append(fwk)
            wlk = []
            for k in range(DCH):
                wk = wlc.tile([P, D], f32r, tag="wlc", name=f"wl0_{k}")
                after_gathers(nc.scalar.dma_start(out=wk[:], in_=wl[0, k]))
                wlk.append(wk)
            at_sb = wbig.tile([P, NCH, N], f32r)
            after_gathers(nc.scalar.dma_start(out=at_sb[:], in_=at_w[:]))
            wih_sb = wbig.tile([P, DCH, K3], f32r)

            # type embeddings: PE-transpose [n, td] -> [td, n]
            for q in range(NCH):
                pt = psA.tile([P, P], f32, tag="psA")
                nc.tensor.transpose(
                    out=pt[:], in_=typg[:, q, :], identity=identity[:]
                )
                nc.vector.tensor_copy(
                    out=fusedT[0][:, q * P : (q + 1) * P], in_=pt[:]
                )

            # token pooling: PE matmul pools 128 tokens -> 32 nodes and
            # transposes to feature-major in one pass
            for s in range(GS):
                tg = gath[s]
                for c2 in range(GC):
                    c = s * GC + c2
                    if pool_wide:
                        for f in range(DCH):
                            pc = psA.tile([P, NF], f32, tag="psA")
                            nc.tensor.matmul(
                                out=pc[:],
                                lhsT=tg[:, c2, f * P : (f + 1) * P],
                                rhs=pool_sb[:, c, :],
                                start=True,
                                stop=True,
                            )
                            if c == 0:
                                nc.vector.tensor_copy(out=fusedT[1 + f][:], in_=pc[:])
                            else:
                                nc.vector.tensor_add(
                                    out=fusedT[1 + f][:],
                                    in0=fusedT[1 + f][:],
                                    in1=pc[:],
                                )
                    else:
                        pc = psA.tile([P, DCH * BLK], f32, tag="psA")
                        for f in range(DCH):
                            nc.tensor.matmul(
                                out=pc[:, f * BLK : (f + 1) * BLK],
                                lhsT=tg[:, c2, f * P : (f + 1) * P],
                                rhs=pool_sb[:, c, :],
                                start=True,
                                stop=True,
                            )
                        for f in range(DCH):
                            nc.vector.tensor_copy(
                                out=fusedT[1 + f][:, c * BLK : (c + 1) * BLK],
                                in_=pc[:, f * BLK : (f + 1) * BLK],
                            )

            # ---- fusion matmul: hT[j] = (fusion_w.T @ fusedT)[j] + b ----
            hT = []
            for j in range(DCH):
                pf = psA.tile([P, NF], f32, tag="psA")
                for k in range(FCH):
                    nc.tensor.matmul(
                        out=pf[:],
                        lhsT=fw[k][:, j * P : (j + 1) * P],
                        rhs=fusedT[k][:],
                        start=(k == 0),
                        stop=(k == FCH - 1),
                    )
                hj = hpool.tile([P, NF], f32r, tag="hpool")
                nc.scalar.activation(
                    out=hj[:], in_=pf[:], func=Ident, bias=fb_sb[:, j : j + 1]
                )
                hT.append(hj)
                after_gathers(nc.scalar.dma_start(out=wih_sb[:, j, :], in_=wih[:, j, :]))

            # ---- GGNN layers ----
            for l in range(L):
                # m = h @ Wl   (node-major out, [node 128, 768] per chunk)
                if l > 0:
                    wlk = []
                    for k in range(DCH):
                        wk = wlc.tile([P, D], f32r, tag="wlc", name=f"wl{l}_{k}")
                        nc.scalar.dma_start(out=wk[:], in_=wl[l, k])
                        wlk.append(wk)
                m_sb = []
                for i in range(NCH):
                    pma = psA.tile([P, NF], f32, tag="psA")
                    pmb = psA.tile([P, D - NF], f32, tag="psA")
                    for k in range(DCH):
                        nc.tensor.matmul(
                            out=pma[:],
                            lhsT=hT[k][:, i * P : (i + 1) * P],
                            rhs=wlk[k][:, :NF],
                            start=(k == 0),
                            stop=(k == DCH - 1),
                        )
                        nc.tensor.matmul(
                            out=pmb[:],
                            lhsT=hT[k][:, i * P : (i + 1) * P],
                            rhs=wlk[k][:, NF:D],
                            start=(k == 0),
                            stop=(k == DCH - 1),
                        )
                    mi = t768.tile([P, D], f32r, tag="t768", name=f"m{l}_{i}")
                    nc.vector.tensor_copy(out=mi[:, :NF], in_=pma[:])
                    nc.vector.tensor_copy(out=mi[:, NF:D], in_=pmb[:])
                    m_sb.append(mi)

                # aggT = m.T @ A.T  (feature-major [feat 128, nodes 512])
                aggT = []
                for j in range(DCH):
                    pa = psA.tile([P, NF], f32, tag="psA")
                    for k in range(NCH):
                        nc.tensor.matmul(
                            out=pa[:],
                            lhsT=m_sb[k][:, j * P : (j + 1) * P],
                            rhs=at_sb[:, k, :],
                            start=(k == 0),
                            stop=(k == NCH - 1),
                        )
                    aj = c512.tile([P, NF], f32r, tag="c512", name=f"agg{l}_{j}")
                    nc.vector.tensor_copy(out=aj[:], in_=pa[:])
                    aggT.append(aj)

                # GRU gates, 128 gate rows at a time
                hnew = []
                for i in range(DCH):
                    # streamed Whh chunks for the three gates at row-chunk i
                    wch = []
                    for g in range(3):
                        w = wst.tile([P, DCH, P], f32r, tag="wst",
                                     name=f"wch{l}_{i}_{g}")
                        wdma = nc.sync.dma_start(out=w[:], in_=whh_st[g * DCH + i])
                        if l == 0 and i == 0:
                            after_gathers(wdma)
                        wch.append(w)

                    # r and z: psum accumulates gi + gh, ACT adds bias+sigmoid
                    rz = []
                    for g in range(2):
                        pg = psA.tile([P, NF], f32, tag="psA")
                        col = g * D + i * P
                        for k in range(DCH):
                            nc.tensor.matmul(
                                out=pg[:],
                                lhsT=wih_sb[:, k, col : col + P],
                                rhs=aggT[k][:],
                                start=(k == 0),
                                stop=False,
                            )
                        for k in range(DCH):
                            nc.tensor.matmul(
                                out=pg[:],
                                lhsT=wch[g][:, k, :],
                                rhs=hT[k][:],
                                start=False,
                                stop=(k == DCH - 1),
                            )
                        gs = gpool.tile([P, NF], f32, tag="gpool",
                                        name=f"g{l}_{i}_{g}")
                        nc.scalar.activation(
                            out=gs[:],
                            in_=pg[:],
                            func=Sigmoid,
                            bias=bsum_sb[:, g * DCH + i : g * DCH + i + 1],
                        )
                        rz.append(gs)
                    r_sb, z_sb = rz

                    # n gate: keep gi and gh separate
                    col = 2 * D + i * P
                    pgin = psA.tile([P, NF], f32, tag="psA")
                    for k in range(DCH):
                        nc.tensor.matmul(
                            out=pgin[:],
                            lhsT=wih_sb[:, k, col : col + P],
                            rhs=aggT[k][:],
                            start=(k == 0),
                            stop=(k == DCH - 1),
                        )
                    pghn = psA.tile([P, NF], f32, tag="psA")
                    for k in range(DCH):
                        nc.tensor.matmul(
                            out=pghn[:],
                            lhsT=wch[2][:, k, :],
                            rhs=hT[k][:],
                            start=(k == 0),
                            stop=(k == DCH - 1),
                        )
                    hb = gpool.tile([P, NF], f32, tag="gpool")
                    nc.scalar.activation(
                        out=hb[:], in_=pghn[:], func=Ident,
                        bias=bhhn_sb[:, i : i + 1],
                    )
                    rn = gpool.tile([P, NF], f32, tag="gpool")
                    nc.vector.tensor_mul(out=rn[:], in0=r_sb[:], in1=hb[:])
                    tn = gpool.tile([P, NF], f32, tag="gpool")
                    nc.vector.tensor_add(out=tn[:], in0=pgin[:], in1=rn[:])
                    nn_ = gpool.tile([P, NF], f32, tag="gpool")
                    nc.scalar.activation(
                        out=nn_[:], in_=tn[:], func=Tanh,
                        bias=bihn_sb[:, i : i + 1],
                    )
                    # h' = n + z * (h - n)
                    s_ = gpool.tile([P, NF], f32, tag="gpool")
                    nc.vector.tensor_sub(out=s_[:], in0=hT[i][:], in1=nn_[:])
                    sz = gpool.tile([P, NF], f32, tag="gpool")
                    nc.vector.tensor_mul(out=sz[:], in0=z_sb[:], in1=s_[:])
                    hj = hpool.tile([P, NF], f32r, tag="hpool",
                                    name=f"h{l}_{i}")
                    nc.vector.tensor_add(out=hj[:], in0=nn_[:], in1=sz[:])
                    hnew.append(hj)
                hT = hnew

            # ---- transpose back to node-major, mask, write out ----
            for i in range(NCH):
                poa = psA.tile([P, NF], f32, tag="psA")
                pob = psA.tile([P, D - NF], f32, tag="psA")
                for j in range(DCH):
                    dst = poa[:, j * P : (j + 1) * P] if j < 4 else \
                        pob[:, (j - 4) * P : (j - 3) * P]
                    nc.tensor.transpose(
                        out=dst,
                        in_=hT[j][:, i * P : (i + 1) * P].bitcast(f32),
                        identity=identity[:],
                    )
                ob = t768.tile([P, D], f32, tag="t768")
                nc.vector.tensor_scalar_mul(
                    out=ob[:, :NF], in0=poa[:], scalar1=mask_sb[:, i : i + 1]
                )
                nc.vector.tensor_scalar_mul(
                    out=ob[:, NF:D], in0=pob[:], scalar1=mask_sb[:, i : i + 1]
                )
                nc.sync.dma_start(out=out[i * P : (i + 1) * P, :], in_=ob[:])

    nc.compile()
    return nc


@functools.lru_cache(maxsize=2)
def _get_nc(pool_wide: bool) -> bass.Bass:
    return build_nc(pool_wide)


def _prep_shared(inputs):
    """Weight tensors identical across graphs, pre-laid-out partition-major."""
    fusion_w = np.ascontiguousarray(np.asarray(inputs["fusion_w"], np.float32))
    fusion_b = np.ascontiguousarray(
        np.asarray(inputs["fusion_b"], np.float32).reshape(DCH, P).T
    )
    wl = np.ascontiguousarray(
        np.asarray(inputs["ggnn_w"], np.float32).reshape(L, DCH, P, D)
    )
    wih_w = np.asarray(inputs["gru_w_ih"], np.float32)   # [K3, D]
    whh_w = np.asarray(inputs["gru_w_hh"], np.float32)
    bih = np.asarray(inputs["gru_b_ih"], np.float32)
    bhh = np.asarray(inputs["gru_b_hh"], np.float32)
    # wih: [P, DCH, K3]  (partition p, feat chunk k -> gate rows)
    wihT = wih_w.T                                       # [D, K3]
    wih = np.ascontiguousarray(wihT.reshape(DCH, P, K3).transpose(1, 0, 2))
    # whh chunks: [GCH, P, DCH, P]
    whhT = whh_w.T                                       # [D, K3]
    whh_st = np.ascontiguousarray(
        np.stack(
            [
                whhT[:, j * P : (j + 1) * P].reshape(DCH, P, P).transpose(1, 0, 2)
                for j in range(GCH)
            ]
        )
    )
    bsum = np.ascontiguousarray((bih + bhh).reshape(GCH, P).T)
    bihn = np.ascontiguousarray(bih[2 * D :].reshape(DCH, P).T)
    bhhn = np.ascontiguousarray(bhh[2 * D :].reshape(DCH, P).T)
    word_emb = np.ascontiguousarray(np.asarray(inputs["word_emb"], np.float32))
    type_table = np.ascontiguousarray(np.asarray(inputs["type_table"], np.float32))
    return dict(
        word_emb=word_emb, type_table=type_table, fusion_w=fusion_w,
        fusion_b=fusion_b, wl=wl, wih=wih, whh_st=whh_st, bsum=bsum,
        bihn=bihn, bhhn=bhhn,
    )


def _graph_blockable(inputs, b):
    seg = np.asarray(inputs["token_seg_ids"][b], np.int64)
    tcol = np.arange(T) // P
    return bool(np.all((seg >= tcol * BLK) & (seg < (tcol + 1) * BLK)))


def _prep_graph(inputs, b, pool_wide):
    tok = np.asarray(inputs["node_token_ids"][b], np.int64)
    typ = np.asarray(inputs["node_types"][b], np.int32)
    seg = np.asarray(inputs["token_seg_ids"][b], np.int64)
    lens = np.asarray(inputs["node_token_lens"][b], np.float64)
    glen = int(np.asarray(inputs["graph_node_lens"][b]))
    esrc = np.asarray(inputs["edge_src"][b], np.int64)
    edst = np.asarray(inputs["edge_dst"][b], np.int64)
    ew = np.asarray(inputs["edge_weight"][b], np.float32)

    # token idxs for dma_gather: GS splits of GT idxs, each wrapped into
    # 16 partitions ([p, s] = idx[s*16+p]) and replicated to 128 partitions
    tok16 = tok.astype(np.int16)
    cols = []
    for s in range(GS):
        w16 = tok16[s * GT : (s + 1) * GT].reshape(GT // 16, 16).T  # [16, GT/16]
        cols.append(np.tile(w16, (8, 1)))                           # [128, GT/16]
    tok_idx = np.ascontiguousarray(np.concatenate(cols, axis=1))    # [128, GS*32]

    typ16 = typ.astype(np.int16)
    typ_idx = np.ascontiguousarray(
        np.tile(typ16.reshape(N // 16, 16).T, (8, 1))
    )                                                               # [128, 32]

    # dense transposed adjacency: AT[src, dst], laid out [P, NCH, N]
    at = np.zeros((N, N), np.float32)
    np.add.at(at, (esrc, edst), ew)
    at = np.ascontiguousarray(at.reshape(NCH, P, N).transpose(1, 0, 2))

    # pooling matrix (1/len weights), [P, TCH, BLK or N]
    winv = np.zeros(N, np.float64)
    nzmask = lens != 0
    winv[nzmask] = 1.0 / lens[nzmask]
    tcol = np.arange(T) // P  # token chunk of each token
    if pool_wide:
        poolm = np.zeros((TCH, P, N), np.float32)
        poolm[tcol, np.arange(T) % P, seg] = winv[seg]
    else:
        poolm = np.zeros((TCH, P, BLK), np.float32)
        poolm[tcol, np.arange(T) % P, seg - tcol * BLK] = winv[seg]
    poolm = np.ascontiguousarray(poolm.transpose(1, 0, 2))

    keep = min(glen, MAX_NODE_LEN)
    mask = np.ascontiguousarray(
        (np.arange(N) < keep).astype(np.float32).reshape(NCH, P).T
    )
    return dict(tok_idx=tok_idx, typ_idx=typ_idx, at_w=at, poolm=poolm,
                maskc=mask)


def kernel(**inputs) -> np.ndarray:
    shared = _prep_shared(inputs)
    pool_wide = not all(_graph_blockable(inputs, b) for b in range(B))
    per_graph = [_prep_graph(inputs, b, pool_wide) for b in range(B)]
    nc = _get_nc(pool_wide)
    in_maps = [{**shared, **per_graph[b]} for b in range(B)]
    res = bass_utils.run_bass_kernel_spmd(nc, in_maps, core_ids=list(range(B)))
    global _last_exec_ns
    _last_exec_ns = res.exec_time_ns
    out = np.stack([r["out"] for r in res.results]).astype(np.float32)
    return out


_last_exec_ns = None
